# revision 14
# baseline (speedup 1.0000x reference)
"""Cosine-similarity retrieval kernel for Trainium2 (8 NeuronCores, SPMD).

Computes out[q, n] = cos(query[q], support[n]) for query [2048, 512] and
support [50000, 512], out [2048, 50000] float32 — matching
torch.nn.CosineSimilarity semantics (dots / max(|q|*|s|, 1e-8)).

Strategy:
  * Shard support on the N axis: 8 shards of 6250 rows (zero-padded to 6272 =
    49 blocks of 128). Each core reads its shard plus the replicated query
    set and writes its own [6272, 2048] output block (n-major, i.e. the
    transpose of the final layout); the host trims/transposes/concatenates —
    no device collective needed.
  * Rows are pre-normalized on the host (norms in float64), so the device
    kernel is a pure matmul; the PSUM result IS the cosine.
  * Storage/matmul dtype is fp16 (1 cycle/row on the PE, same as fp32r, but
    weights go through the LDWEIGHTS+FWL path instead of per-matmul fp32
    self-loading). The support block [128d, 128n] is the STATIONARY operand,
    reused across 4 consecutive matmuls that stream the resident query set
    512 columns at a time; with walrus --enable-ldw-opt the LDWEIGHTS for
    repeats is deduped, so weight-load overhead amortizes 4x and prefetches
    into the PE background buffer during the preceding matmuls.
  * PSUM: 4 banks accumulate one n-block over the 4 k-slices (bank = [128,
    512] fp32 = exactly one 2KB bank); the other 4 banks drain the previous
    n-block through ACT/DVE fp32->fp16 copies, so the PE never waits.
  * Output staged fp16 (halves the dominant HBM write traffic; host upcasts;
    ~2.4e-4 extra rel-L2). One store per n-block: 4KB-contiguous per
    partition, and the final store is only 0.5MB so the kernel-exit barrier
    isn't stuck behind a big trailing DMA.
"""

import os

import numpy as np

QN, DN, NN = 2048, 512, 50000
N_CORES = 8
NSH = NN // N_CORES  # 6250 support rows per core
P = 128
KT = DN // P  # 4 contraction slices
NBLK = (NSH + P - 1) // P  # 49 n-blocks per core
NSHP = NBLK * P  # 6272 (22 zero-padded rows, trimmed on host)
QC = 4  # query chunks, each one PSUM bank wide
QW = QN // QC  # 512 fp32 = one full PSUM bank
# n-blocks per DMA slab: small first slab so the first matmul unblocks after
# ~0.3MB of DMA; 1MB slabs after that for 2KB-contiguous packets.
SLAB_BLOCKS = [2, 8, 8, 8, 8, 8, 7]
SLAB_PREFETCH = 3
EPS = 1e-8
# PE p-state warmup: the Tensor engine runs at 0.65-1.2GHz until it has been
# continuously active for ~3.4us (free-running activity window), then 2.4GHz.
# The startup DMA wait is ~3us of forced PE idle; fill it with dummy 128-col
# matmuls on a zeroed scratch tile so the real stream starts at full clock.
WARM_MM = int(os.environ.get("COS_WARM_MM", "40"))

# "fp16" (default), "bf16", or "fp32r": SBUF/DMA storage + matmul dtype.
DT_MODE = os.environ.get("COS_DT_MODE", "fp16")
# Output staging dtype: "fp16" (default) or "fp32".
OUT_MODE = os.environ.get("COS_OUT_DT", "fp16")

_PROGRAM = {}


def _patch_ldw_opt():
    """walrus's LDWEIGHTS dedup (--enable-ldw-opt) is hardcoded off in
    concourse; consecutive matmuls here share weights, so turn it on."""
    from concourse import bass_utils as bu

    if getattr(bu.run_command, "_ldw_patched", False):
        return
    orig = bu.run_command

    def patched(argv, **kwargs):
        if isinstance(argv, list) and "--enable-ldw-opt=false" in argv:
            argv = [
                "--enable-ldw-opt=true" if a == "--enable-ldw-opt=false" else a
                for a in argv
            ]
        return orig(argv, **kwargs)

    patched._ldw_patched = True
    bu.run_command = patched


# walrus's NEFF postamble resets the semaphore file between executions by
# having each engine clear its ~51-sem block one EVENT_SEMAPHORE at a time;
# on the Tensor engine that's 53 clears x 129ns = ~6.8us of pure epilogue.
# Measured: --num-semaphores-per-queue=16 does NOT shrink it (the wipe is
# hardcoded in codegen); knob kept for experiments, default off.
SEM_PER_QUEUE = os.environ.get("COS_SEM_PER_QUEUE", "")


def _patch_walrus_sems():
    from concourse import bass_utils as bu

    if getattr(bu.run_command, "_sem_patched", False):
        return
    orig = bu.run_command

    def patched(argv, **kwargs):
        if (
            SEM_PER_QUEUE
            and isinstance(argv, list)
            and any("neff_packager" in str(a) for a in argv)
        ):
            argv = list(argv) + [f"--num-semaphores-per-queue={SEM_PER_QUEUE}"]
        return orig(argv, **kwargs)

    patched._sem_patched = True
    patched._ldw_patched = getattr(orig, "_ldw_patched", False)
    bu.run_command = patched


def _build_program(dt_mode, out_mode):
    import concourse.bass as bass  # noqa: F401
    import concourse.tile as tile
    from concourse import bacc, mybir

    # walrus's LDWEIGHTS dedup pass rejects fp16/bf16 (FWL-format) weight
    # loads outright ("InstLdweights is not compatible with LDW
    # optimization"), so only enable it for the fp32r fallback.  The fp16
    # per-matmul LDWEIGHTS is hidden by FWL + the PE's 64-deep reorder
    # window anyway (measured ~5ns/matmul exposure).
    if dt_mode == "fp32r" and os.environ.get("COS_LDW_OPT", "1") != "0":
        _patch_ldw_opt()
    _patch_walrus_sems()

    store_dt = {
        "fp16": mybir.dt.float16,
        "bf16": mybir.dt.bfloat16,
        "fp32r": mybir.dt.float32r,
    }[dt_mode]
    out_dt = mybir.dt.float16 if out_mode == "fp16" else mybir.dt.float32

    nc = bacc.Bacc(
        "TRN2", target_bir_lowering=False, debug=False, num_devices=N_CORES
    )
    qT = nc.dram_tensor("qT", [DN, QN], store_dt, kind="ExternalInput").ap()
    sT = nc.dram_tensor("sT", [DN, NSHP], store_dt, kind="ExternalInput").ap()
    out = nc.dram_tensor("out", [NSHP, QN], out_dt, kind="ExternalOutput").ap()

    qT3 = qT.rearrange("(k p) q -> p k q", p=P)  # [128, KT, QN]
    sT3 = sT.rearrange("(k p) n -> p k n", p=P)  # [128, KT, NSHP]
    out3 = out.rearrange("(s p) q -> p s q", p=P)  # [128, NBLK, QN]

    slab_off, o = [], 0
    for nb in SLAB_BLOCKS:
        slab_off.append(o)
        o += nb
    assert o == NBLK

    with tile.TileContext(nc) as tc:
        with (
            tc.tile_pool(name="qw", bufs=1) as qpool,
            tc.tile_pool(name="sw", bufs=SLAB_PREFETCH + 1) as spool,
            tc.tile_pool(name="ps", bufs=8, space="PSUM") as pspool,
            tc.tile_pool(name="ostage", bufs=4) as opool,
        ):
            qt = qpool.tile([P, KT, QN], store_dt, name="qt", tag="qt")
            slabs = {}

            def load_slab(si, per_k=False):
                nb = SLAB_BLOCKS[si]
                n0 = slab_off[si] * P
                w = nb * P
                t = spool.tile(
                    [P, KT, 8 * P],
                    store_dt,
                    name=f"s{si}",
                    tag="ss",
                    bufs=SLAB_PREFETCH + 1,
                )
                if per_k:
                    for k in range(KT):
                        nc.sync.dma_start(t[:, k, :w], sT3[:, k, n0 : n0 + w])
                else:
                    nc.sync.dma_start(t[:, :, :w], sT3[:, :, n0 : n0 + w])
                slabs[si] = t

            # PE warmup: dummy matmuls on zeroed scratch, emitted first so
            # they run during the startup DMA wait and ramp the HAM governor
            # to full clock before the first real matmul.
            if WARM_MM:
                wscr = qpool.tile([P, 256], store_dt, name="wscr", tag="wscr")
                nc.vector.memset(wscr[:, :], 0)
                wps = pspool.tile(
                    [P, QW], mybir.dt.float32, name="ps", tag="ps"
                )
                for _ in range(WARM_MM):
                    nc.tensor.matmul(
                        wps[:, :P],
                        lhsT=wscr[:, :P],
                        rhs=wscr[:, P : 2 * P],
                        start=True,
                        stop=True,
                    )

            # Startup: everything stays on the sync queue (spreading over
            # other engines' queues loses: each ring pays its own multi-us
            # cold ramp, and the extra DMA semaphores lengthen the exit
            # drain/clear epilogue).  Each DMA_DIRECT2D descriptor costs
            # ~610ns of serialized issue on Sync, so s0 goes out as ONE
            # all-k descriptor (256KB) followed by the 4 query k-slices the
            # first matmuls are gated on: 5 descriptors instead of 8 pulls
            # the first matmul ~2us earlier.  Measured (prev session):
            # starting the PE on even more partial data always stutters and
            # loses 2-6us net — the dummy-warmup above fills the wait
            # instead.
            nb0 = SLAB_BLOCKS[0]
            w0 = nb0 * P
            t0s = spool.tile(
                [P, KT, 8 * P],
                store_dt,
                name="s0",
                tag="ss",
                bufs=SLAB_PREFETCH + 1,
            )
            # Prime the DMA ring with a tiny transfer: the first descriptor
            # pays ~1.3us of ring cold-start before packets flow; absorbing
            # it on a 1KB dummy (own scratch tile, no deps) lets s0/q0 flow
            # right after their issue.
            if os.environ.get("COS_PRIME_DMA", "1") != "0":
                prime = qpool.tile([P, 4], store_dt, name="prime", tag="prime")
                nc.sync.dma_start(prime[:, :], sT3[:, 0, 0:4])
            nc.sync.dma_start(t0s[:, :, :w0], sT3[:, :, 0:w0])
            for k in range(KT):
                nc.sync.dma_start(qt[:, k, :], qT3[:, k, :])
            slabs[0] = t0s
            for si in range(1, SLAB_PREFETCH):
                load_slab(si)

            copy_idx = 0
            for si, nb in enumerate(SLAB_BLOCKS):
                if si + SLAB_PREFETCH < len(SLAB_BLOCKS):
                    load_slab(si + SLAB_PREFETCH)
                for b in range(nb):
                    sb = slab_off[si] + b
                    last = sb == NBLK - 1
                    if not last:
                        pss = [
                            pspool.tile(
                                [P, QW], mybir.dt.float32, name="ps", tag="ps"
                            )
                            for _ in range(QC)
                        ]
                        ot = opool.tile([P, QN], out_dt, name="ot", tag="ot")
                        # k outer / qc inner: the 4 qc matmuls stream against
                        # one stationary [128, 128] support block, so its
                        # LDWEIGHTS prefetches into the PE background buffer
                        # during the previous k's matmuls.
                        for k in range(KT):
                            wt = slabs[si][:, k, b * P : (b + 1) * P]
                            for qc in range(QC):
                                nc.tensor.matmul(
                                    pss[qc][:, :],
                                    lhsT=wt,
                                    rhs=qt[:, k, qc * QW : (qc + 1) * QW],
                                    start=(k == 0),
                                    stop=(k == KT - 1),
                                )
                        # split PSUM->SBUF downcast copies across ACT/DVE
                        for qc in range(QC):
                            dst = ot[:, qc * QW : (qc + 1) * QW]
                            if copy_idx % 2 == 0:
                                nc.scalar.copy(out=dst, in_=pss[qc][:, :])
                            else:
                                nc.vector.tensor_copy(out=dst, in_=pss[qc][:, :])
                            copy_idx += 1
                        nc.sync.dma_start(out3[:, sb, :], ot[:, :])
                    else:
                        # Last block runs qc-major with a store per qc chunk
                        # (128KB) so the kernel-exit drain only waits on the
                        # final 128KB flight instead of the whole 512KB
                        # block: trims the exposed tail after the last
                        # matmul.  The 3 extra 0.6us store issues overlap
                        # the block's remaining compute on Sync.
                        ot = opool.tile([P, QN], out_dt, name="ot", tag="ot")
                        for qc in range(QC):
                            ps = pspool.tile(
                                [P, QW], mybir.dt.float32, name="ps", tag="ps"
                            )
                            for k in range(KT):
                                wt = slabs[si][:, k, b * P : (b + 1) * P]
                                nc.tensor.matmul(
                                    ps[:, :],
                                    lhsT=wt,
                                    rhs=qt[:, k, qc * QW : (qc + 1) * QW],
                                    start=(k == 0),
                                    stop=(k == KT - 1),
                                )
                            dst = ot[:, qc * QW : (qc + 1) * QW]
                            if qc % 2 == 0:
                                nc.scalar.copy(out=dst, in_=ps[:, :])
                            else:
                                nc.vector.tensor_copy(out=dst, in_=ps[:, :])
                            nc.sync.dma_start(
                                out3[:, sb, qc * QW : (qc + 1) * QW], dst
                            )
    nc.compile()
    return nc


def _get_program(dt_mode=None, out_mode=None):
    key = (dt_mode or DT_MODE, out_mode or OUT_MODE)
    if key not in _PROGRAM:
        _PROGRAM[key] = _build_program(*key)
    return _PROGRAM[key]


def _round_fp32r(x):
    """Round fp32 to the PE's float32r format: round-to-nearest-even keeping
    11 explicit mantissa bits (low 12 bits zeroed)."""
    u = np.ascontiguousarray(x, dtype=np.float32).view(np.uint32)
    lsb = (u >> 12) & 1
    r = (u + np.uint32(0x7FF) + lsb) & np.uint32(0xFFFFF000)
    return r.view(np.float32)


def _host_dt(dt_mode):
    if dt_mode == "fp16":
        return np.float16
    if dt_mode == "fp32r":
        return np.float32
    from ml_dtypes import bfloat16

    return bfloat16


def _prep_inputs(support_set, query_set, dt_mode=None):
    dt_mode = dt_mode or DT_MODE
    S = np.asarray(support_set, dtype=np.float32)
    Q = np.asarray(query_set, dtype=np.float32)
    assert S.shape == (NN, DN) and Q.shape == (QN, DN)
    hdt = _host_dt(dt_mode)

    def normalize(x):
        x64 = x.astype(np.float64)
        norm = np.sqrt(np.einsum("nd,nd->n", x64, x64))
        # Reference divides by max(|q|*|s|, eps). Norms here are ~22, so the
        # eps clamp never binds for real rows; an all-zero row would give
        # dots == 0 in the reference too, so map inv-norm to 0 there.
        inv = np.where(norm > 0, 1.0 / np.maximum(norm, EPS), 0.0)
        return x64 * inv[:, None]

    Sn = normalize(S)
    Qn = normalize(Q)
    qT = np.ascontiguousarray(Qn.T).astype(hdt)  # [512, 2048]
    if dt_mode == "fp32r":
        qT = _round_fp32r(qT)
    in_maps = []
    for c in range(N_CORES):
        sT = np.zeros((DN, NSHP), dtype=hdt)
        sT[:, :NSH] = np.ascontiguousarray(Sn[c * NSH : (c + 1) * NSH].T).astype(
            hdt
        )
        if dt_mode == "fp32r":
            sT = _round_fp32r(sT)
        in_maps.append({"qT": qT, "sT": sT})
    return in_maps


def _run(in_maps, dt_mode=None, out_mode=None, trace=False, **kwargs):
    from concourse import bass_utils

    nc = _get_program(dt_mode, out_mode)
    return bass_utils.run_bass_kernel_spmd(
        nc, in_maps, core_ids=list(range(N_CORES)), trace=trace, **kwargs
    )


def _assemble(results):
    out = np.empty((QN, NN), dtype=np.float32)
    for c in range(N_CORES):
        blk = np.asarray(results[c]["out"])[:NSH]  # [6250, 2048]
        out[:, c * NSH : (c + 1) * NSH] = blk.T
    return out


def kernel(support_set, query_set):
    in_maps = _prep_inputs(support_set, query_set)
    res = _run(in_maps)
    return _assemble(res.results)

